# revision 127
# baseline (speedup 1.0000x reference)
"""CrossAttention Trainium2 kernel — 8-core tensor-parallel (2 heads/core).

Self-contained: builds a Bass/Tile kernel, shards the full inputs across the
8 NeuronCores, runs via the axon PJRT path, and gathers the full output.

Per-core layout (core c owns heads 2c, 2c+1 = 128 of 1024 model dims):
  phase P: q/k/v projections (fp16 matmuls, fp32 PSUM) + rotary applied with
           host-precomputed cos/sin tables; the "spliced" operand comes from a
           partition-pair-swapped SBUF->SBUF DMA copy (sign folded into sin).
  phase A: per (b, qb, h): QK^T scores into 2-bank PSUM tiles, batched Exp on
           the Act engine, PV with q-tokens on PSUM partitions (out free = 65:
           64 channels + a ones-column giving the softmax denominator), then
           normalize on evacuation via a per-partition reciprocal scale.
  phase O: transpose attention output back to [chan, tok] via identity
           matmuls, single-pass output projection, partial fp16 outputs summed
           (+ Wo@bv + bo folded in) on the host.
"""

import sys
import time

for _p in ("/opt/trn_rl_repo", "/root/.axon_site/_ro/trn_rl_repo"):
    if _p not in sys.path:
        sys.path.insert(0, _p)

import numpy as np
from contextlib import ExitStack

import concourse.bacc as bacc
import concourse.mybir as mybir
import concourse.tile as tile
from concourse.mybir import ActivationFunctionType as AF
from concourse.mybir import AluOpType as ALU

# ---------------------------------------------------------------- problem dims
D = 1024
H = 16
DH = 64
TQ = 2048
TKV = 2048
B = 2
NCORES = 8
HPC = H // NCORES          # heads per core = 2
MPC = HPC * DH             # dims per core  = 128
T = B * TQ                 # token axis (b-grouped) = 4096
NROT = 32                  # rotated channels per head (frac 0.5 of 64)
NHEADS_ROT = 12            # rotated heads (frac 0.75 of 16)
MAX_WL = 8192.0

F32 = mybir.dt.float32
F16 = mybir.dt.float16
NPF16 = np.float16

KC = D // 128              # 8 contraction chunks for projections
TB = T // 512              # 8 token blocks of 512
TBB = TQ // 512            # 4 token blocks per batch
QB = TQ // 512             # 4 query blocks per batch
KVC = TKV // 128           # 16 kv chunks per batch
VW = 2 * (DH + 1)          # vn chunk width: [h0 ch, ones, h1 ch, ones] = 130


# ---------------------------------------------------------------- bass builder
def build_kernel(use_mask: bool):
    nc = bacc.Bacc("TRN2", target_bir_lowering=False, debug=False,
                   enable_asserts=True, num_devices=NCORES)

    xqT = nc.dram_tensor("xqT", [D, T], F16, kind="ExternalInput").ap()
    xkvT = nc.dram_tensor("xkvT", [D, T], F16, kind="ExternalInput").ap()
    # w3 packs [wkT | wvT | wqT] so the whole projection weight set is one
    # DMA; bqk packs [bk | bq]; tabq/tabk pack [cos_b0|sin_b0|cos_b1|sin_b1]
    # so each batch-half of a rotary table is one DMA. Fewer startup DMAs
    # matter: issue overhead is ~1.25us per DMA on an in-order queue.
    w3_d = nc.dram_tensor("w3", [D, 3 * MPC], F16, kind="ExternalInput").ap()
    bqk_d = nc.dram_tensor("bqk", [MPC, 2], F32, kind="ExternalInput").ap()
    woT_d = nc.dram_tensor("woT", [MPC, D], F16, kind="ExternalInput").ap()
    tabq_d = nc.dram_tensor("tabq", [128, 2 * T], F16,
                            kind="ExternalInput").ap()
    tabk_d = nc.dram_tensor("tabk", [128, 2 * T], F16,
                            kind="ExternalInput").ap()
    idpm_d = nc.dram_tensor("idpm", [128, 256], F16,
                            kind="ExternalInput").ap()
    if use_mask:
        mask_d = nc.dram_tensor("maskT", [TKV, B, TQ], F16,
                                kind="ExternalInput").ap()
    outT = nc.dram_tensor("outT", [D, T], F16, kind="ExternalOutput").ap()

    with tile.TileContext(nc) as tc:
        with ExitStack() as octx:
            persist = octx.enter_context(tc.tile_pool(name="persist", bufs=1))
            xio = octx.enter_context(tc.tile_pool(name="xio", bufs=2))
            rotp = octx.enter_context(tc.tile_pool(name="rotp", bufs=4))
            ptp = octx.enter_context(
                tc.tile_pool(name="ptp", bufs=12 if use_mask else 16))
            smlp = octx.enter_context(tc.tile_pool(name="smlp", bufs=8))
            otqp = octx.enter_context(tc.tile_pool(name="otqp", bufs=16))
            osb = octx.enter_context(tc.tile_pool(name="osb", bufs=9))
            pjp = octx.enter_context(tc.tile_pool(name="pjp", bufs=2,
                                                  space="PSUM"))
            stp = octx.enter_context(tc.tile_pool(name="stp", bufs=2,
                                                  space="PSUM"))
            pvp = octx.enter_context(tc.tile_pool(name="pvp", bufs=2,
                                                  space="PSUM"))
            if use_mask:
                mkp = octx.enter_context(tc.tile_pool(name="mkp", bufs=4))

            # -------- persistent tensors
            qT = persist.tile([128, T], F16, tag="qT")
            kT = persist.tile([128, T], F16, tag="kT")
            ot_t = persist.tile([128, T], F16, tag="ot_t")
            tabq = persist.tile([128, 2 * T], F16, tag="tabq")
            tabk = persist.tile([128, 2 * T], F16, tag="tabk")
            # vn chunk layout [ones | h0 chans | h1 chans | ones] so v-evac is
            # a single [128,128] copy and both heads' PV moving APs (chans +
            # their ones column) stay contiguous.
            vn = persist.tile([128, B * KVC * VW], F16, tag="vn")
            vn3 = vn.rearrange("p (c w) -> p c w", w=VW)
            woT = persist.tile([MPC, D], F16, tag="woT")
            idpm = persist.tile([128, 256], F16, tag="idpm")
            bqk_sb = persist.tile([MPC, 2], F32, tag="bqk")
            w3_all = persist.tile([128, KC, 3 * MPC], F16, tag="w3_all")

            # Startup loads in last-needed order on SP; the tiny bias load
            # goes on Act whose queue otherwise handles only the startup
            # rotary splices. Transfer completion order ~= this issue order
            # on the shared DMA engines.
            xq3 = xqT.rearrange("(c p) t -> p c t", p=128)
            xkv3 = xkvT.rearrange("(c p) t -> p c t", p=128)
            nc.sync.dma_start(w3_all[:],
                              w3_d.rearrange("(c p) m -> p c m", p=128))
            nc.scalar.dma_start(bqk_sb[:], bqk_d[:])
            nc.scalar.dma_start(idpm[:], idpm_d[:])
            pre01kv = xio.tile([128, KC, 1024], F16, tag="xkv01",
                               name="xkv_t01", bufs=1)
            nc.sync.dma_start(pre01kv[:], xkv3[:, :, 0:1024])
            nc.sync.dma_start(tabk[:, 0:2048], tabk_d[:, 0:2048])
            pre0q = xio.tile([128, KC, 512], F16, tag="xq", name="xq_t0")
            nc.scalar.dma_start(pre0q[:], xq3[:, :, 0:512])
            nc.scalar.dma_start(tabq[:, 0:1024], tabq_d[:, 0:1024])
            nc.scalar.dma_start(tabq[:, 1024:4096], tabq_d[:, 1024:4096])
            pre2kv = xio.tile([128, KC, 512], F16, tag="xkv", name="xkv_t2",
                              bufs=3)
            nc.sync.dma_start(pre2kv[:], xkv3[:, :, 1024:1536])
            nc.vector.memset(vn3[:, :, 0:1], 1.0)
            nc.vector.memset(vn3[:, :, 2 * DH + 1:2 * DH + 2], 1.0)
            wk_sb = [w3_all[:, kc, 0:MPC] for kc in range(KC)]
            wv_sb = [w3_all[:, kc, MPC:2 * MPC] for kc in range(KC)]
            wq_sb = [w3_all[:, kc, 2 * MPC:3 * MPC] for kc in range(KC)]
            bk_sb = bqk_sb[:, 0:1]
            bq_sb = bqk_sb[:, 1:2]

            # -------- emit helpers ------------------------------------------
            def _tab_aps(tab, tb):
                # block-interleaved layout: [cos_blk | sin_blk] per 512 tokens
                base = tb * 1024
                return (tab[:, base:base + 512],
                        tab[:, base + 512:base + 1024])

            def _rotary(xt, tab, tb, dma=None, perm=False):
                """x = x*cos + pairswap(x)*sin, splice sign folded into the
                sin table. The swap is an SBUF->SBUF DMA in steady state; on
                the startup critical path (perm=True) it is a PE permutation
                matmul instead, avoiding the serialized SP DMA queue."""
                sl = slice(tb * 512, (tb + 1) * 512)
                cs, sn = _tab_aps(tab, tb)
                sh = rotp.tile([128, 512], F16, tag="sh", name="sh")
                if perm:
                    pps = pjp.tile([128, 512], F32, tag="pj", name="pps")
                    nc.tensor.matmul(pps[:], idpm[:, 128:256],
                                     xt[:, sl], start=True, stop=True)
                    nc.vector.tensor_mul(sh[:], pps[:], sn)
                else:
                    dma = dma or nc.sync
                    dma.dma_start(sh[0:127:2, :], xt[1:128:2, sl])
                    dma.dma_start(sh[1:128:2, :], xt[0:127:2, sl])
                    nc.vector.tensor_mul(sh[:], sh[:], sn)
                nc.vector.tensor_mul(xt[:, sl], xt[:, sl], cs)
                nc.vector.tensor_add(xt[:, sl], xt[:, sl], sh[:])

            xkv_tiles = {}

            def _xkv(tb, pre=None):
                if tb not in xkv_tiles:
                    if pre is not None:
                        xkv_tiles[tb] = pre
                    else:
                        t = xio.tile([128, KC, 512], F16, tag="xkv",
                                     name="xkv_t", bufs=3)
                        nc.sync.dma_start(t[:],
                                          xkv3[:, :, tb * 512:(tb + 1) * 512])
                        xkv_tiles[tb] = t
                return xkv_tiles[tb]

            def kvk_part(tb, pre=None, dma=None, perm=False, ea=False):
                """k projection + k rotary for one 512-token block."""
                sl = slice(tb * 512, (tb + 1) * 512)
                xkv_t = _xkv(tb, pre)
                k_ps = pjp.tile([128, 512], F32, tag="pj", name="k_ps")
                for kc in range(KC):
                    nc.tensor.matmul(k_ps[:], wk_sb[kc], xkv_t[:, kc, :],
                                     start=(kc == 0), stop=(kc == KC - 1))
                if ea:   # startup blocks: evac on Act (idle) to keep DVE's
                    nc.scalar.activation(kT[:, sl], k_ps[:], AF.Identity,
                                         bias=bk_sb)
                else:    # in-order queue clear for the rotary muls
                    nc.vector.tensor_scalar(kT[:, sl], k_ps[:], bk_sb, None,
                                            ALU.add)
                _rotary(kT, tabk, tb, dma, perm)

            def kvv_part(tb, evac_act=False):
                """v projection for one block; natural [tok, chan] layout."""
                xkv_t = _xkv(tb)
                v_ps = pjp.tile([128, 512], F32, tag="pj", name="v_ps")
                for tc4 in range(4):
                    vsl = slice(tc4 * 128, (tc4 + 1) * 128)
                    for kc in range(KC):
                        nc.tensor.matmul(v_ps[:, vsl], xkv_t[:, kc, vsl],
                                         wv_sb[kc], start=(kc == 0),
                                         stop=(kc == KC - 1))
                for tc4 in range(4):
                    g = tb * 4 + tc4
                    dst = vn3[:, g, 1:2 * DH + 1]
                    src = v_ps[:, tc4 * 128:(tc4 + 1) * 128]
                    if evac_act:
                        nc.scalar.activation(dst, src, AF.Identity)
                    else:
                        nc.vector.tensor_copy(dst, src)
                del xkv_tiles[tb]

            def kv_part(tb, pre=None, dma=None, perm=False, ea=False):
                kvk_part(tb, pre, dma, perm, ea)
                kvv_part(tb)

            def q_part(tb, pre=None, dma=None, perm=False, ea=False):
                """q projection + rotary for one 512-token block."""
                sl = slice(tb * 512, (tb + 1) * 512)
                if pre is None:
                    xq_t = xio.tile([128, KC, 512], F16, tag="xq",
                                    name="xq_t")
                    nc.sync.dma_start(xq_t[:], xq3[:, :, sl])
                else:
                    xq_t = pre
                q_ps = pjp.tile([128, 512], F32, tag="pj", name="q_ps")
                for kc in range(KC):
                    nc.tensor.matmul(q_ps[:], wq_sb[kc], xq_t[:, kc, :],
                                     start=(kc == 0), stop=(kc == KC - 1))
                if ea:
                    nc.scalar.activation(qT[:, sl], q_ps[:], AF.Identity,
                                         bias=bq_sb)
                else:
                    nc.vector.tensor_scalar(qT[:, sl], q_ps[:], bq_sb, None,
                                            ALU.add)
                _rotary(qT, tabq, tb, dma, perm)

            otq_tiles = {}

            def qk_exp(b, qb, h, halves=None, pts=None):
                """scores + exp for one (batch, 512-query-block, head)."""
                hsl = slice(h * DH, (h + 1) * DH)
                qsl = slice(b * TQ + qb * 512, b * TQ + (qb + 1) * 512)
                mts = mask_tiles.get((b, qb)) if use_mask else None
                if pts is None:
                    pts = []
                for half in (range(KVC // 2) if halves is None else halves):
                    st = stp.tile([128, 1024], F32, tag="st", name="st")
                    for j in range(2):
                        kc = half * 2 + j
                        nc.tensor.matmul(
                            st[:, j * 512:(j + 1) * 512],
                            kT[hsl, b * TKV + kc * 128:b * TKV + (kc + 1) * 128],
                            qT[hsl, qsl], start=True, stop=True)
                    pt = ptp.tile([128, 1024], F16, tag="pt", name="pt")
                    nc.scalar.activation(pt[:], st[:], AF.Exp)
                    if use_mask:
                        nc.vector.tensor_mul(pt[:], pt[:], mts[half][:])
                    pts.append(pt)
                return pts

            def pv_norm(b, qb, h, pts):
                """PV with q on partitions, denominator col, normalize+evac."""
                ov = pvp.tile([128, 4 * (DH + 1)], F32, tag="pv", name="ov")
                for qt in range(4):
                    osl = slice(qt * (DH + 1), (qt + 1) * (DH + 1))
                    for kc in range(KVC):
                        pcol = (kc % 2) * 512 + qt * 128
                        nc.tensor.matmul(
                            ov[:, osl],
                            pts[kc // 2][:, pcol:pcol + 128],
                            vn3[:, b * KVC + kc, h * (DH + 1):
                                (h + 1) * (DH + 1)],
                            start=(kc == 0), stop=(kc == KVC - 1))
                s_off = 0 if h == 0 else DH       # ones col position per head
                c_off = 1 if h == 0 else 0
                for qt in range(4):
                    base = qt * (DH + 1)
                    rec = smlp.tile([128, 1], F32, tag="rec", name="rec")
                    nc.vector.reciprocal(
                        rec[:], ov[:, base + s_off:base + s_off + 1])
                    otq = otqp.tile([128, DH], F16, tag="otq", name="otq")
                    nc.vector.tensor_scalar(
                        otq[:], ov[:, base + c_off:base + c_off + DH],
                        rec[:], None, ALU.mult)
                    otq_tiles[(h, qt)] = otq

            def transposes(b, qb):
                """[q, chan] -> ot_t[chan, tok] via identity matmuls."""
                for qt in range(4):
                    tr = pvp.tile([128, 4 * (DH + 1)], F32, tag="pv",
                                  name="tr")
                    for h in range(HPC):
                        nc.tensor.matmul(tr[h * DH:(h + 1) * DH, 0:128],
                                         otq_tiles[(h, qt)][:],
                                         idpm[:, 0:128],
                                         start=True, stop=True)
                    col = b * TQ + qb * 512 + qt * 128
                    nc.vector.tensor_copy(ot_t[:, col:col + 128],
                                          tr[:, 0:128])

            osb_tiles = {}

            def outproj_quarter(b, jc, tb4, evac_act=False):
                """one [128,512] token-quarter of output row-block jc."""
                key = (b, jc)
                if key not in osb_tiles:
                    osb_tiles[key] = osb.tile([128, 2048], F16, tag="o_sb",
                                              name="o_sb")
                o_big = osb_tiles[key]
                o_ps = pjp.tile([128, 512], F32, tag="pj", name="o_ps")
                col = b * TQ + tb4 * 512
                nc.tensor.matmul(o_ps[:], woT[:, jc * 128:(jc + 1) * 128],
                                 ot_t[:, col:col + 512], start=True, stop=True)
                c0 = tb4 * 512
                if evac_act and jc % 2 == 0:
                    # tail quarters: split evacs between Act (idle after the
                    # last exp) and DVE so neither serializes the tail
                    nc.scalar.activation(o_big[:, c0:c0 + 512], o_ps[:],
                                         AF.Identity)
                else:
                    nc.vector.tensor_copy(o_big[:, c0:c0 + 512], o_ps[:])
                dma = nc.sync if (evac_act and jc % 2 == 1) else nc.gpsimd
                dma.dma_start(
                    outT[jc * 128:(jc + 1) * 128,
                         b * TQ + c0:b * TQ + c0 + 512],
                    o_big[:, c0:c0 + 512])

            mask_tiles = {}

            def load_mask(b, qb):
                if not use_mask or (b, qb) in mask_tiles:
                    return
                mts = []
                for half in range(KVC // 2):
                    mt = mkp.tile([128, 1024], F16, tag="mk", name="mt")
                    for j in range(2):
                        kc = half * 2 + j
                        nc.sync.dma_start(
                            mt[:, j * 512:(j + 1) * 512],
                            mask_d[kc * 128:(kc + 1) * 128, b,
                                   qb * 512:(qb + 1) * 512])
                    mts.append(mt)
                mask_tiles[(b, qb)] = mts

            # -------- schedule ----------------------------------------------
            # kv-side of b0 projects first and the first combo's QK halves
            # interleave with the remaining kv blocks, so exp starts as soon
            # as kv blocks 0-1 + q block 0 are rotated. The remaining q/kv
            # blocks and the output projection are interleaved into the
            # attention pipeline as PE fillers inside Act-bound exp windows.
            # One-combo-deep software pipeline: QK+exp of combo i issues
            # before PV of combo i-1. DMAs are emitted just-in-time so SP's
            # in-order queue feeds the startup critical path first.
            # critical path to the first exp: kvk0, q0, kvk1 projections +
            # rotaries (splices on Act's queue), QK halves as kv blocks land;
            # v-projections are deferred into the early exp windows.
            kv_part(0, pre=pre01kv[:, :, 0:512], ea=True)
            kv_part(1, pre=pre01kv[:, :, 512:1024], ea=True)
            q_part(0, pre=pre0q, ea=True)
            nc.sync.dma_start(tabk[:, 2048:4096], tabk_d[:, 2048:4096])
            load_mask(0, 0)
            kv_part(2, pre=pre2kv)
            pts0 = qk_exp(0, 0, 0, halves=[0, 1, 2, 3])
            qk_exp(0, 0, 0, halves=[4, 5], pts=pts0)
            kv_part(3)
            nc.sync.dma_start(tabk[:, 4096:8192], tabk_d[:, 4096:8192])
            qk_exp(0, 0, 0, halves=[6, 7], pts=pts0)
            nc.sync.dma_start(tabq[:, 4096:8192], tabq_d[:, 4096:8192])
            nc.sync.dma_start(woT[:], woT_d[:])

            combos = [(b, qb, h) for b in range(B) for qb in range(QB)
                      for h in range(HPC)]
            fillers = {
                (0, 0, 1): [("q", 1)],
                (0, 1, 0): [("kv", 4)], (0, 1, 1): [("q", 2)],
                (0, 2, 0): [("kv", 5)], (0, 2, 1): [("q", 3), ("q", 4)],
                (0, 3, 0): [("kv", 6)], (0, 3, 1): [("kv", 7)],
                (1, 0, 0): [("q", 5)],
                (1, 1, 0): [("q", 6)],
                (1, 2, 0): [("q", 7)],
            }
            # outproj quarters: (0,*) available from combo index 9 (after
            # transposes(0,3) at index 8); (1,*,tb4) needs transposes(1,tb4),
            # emitted during combo index 10+2*tb4.
            opq = {9: [(0, 0), (0, 1)], 10: [(0, 2), (0, 3)],
                   11: [(0, 4), (0, 5)], 12: [(0, 6), (0, 7)]}
            for i, jcs in opq.items():
                fillers.setdefault(combos[i], []).extend(
                    ("op", b, jc, tb4) for b, jc in jcs for tb4 in range(4))
            h0slots = {13: [0, 1, 2], 14: [3, 4, 5], 15: [6, 7]}
            for i, jcs in h0slots.items():
                fillers.setdefault(combos[i], []).extend(
                    ("op", 1, jc, tb4) for jc in jcs for tb4 in (0, 1))
            # t2 quarters fit in the final exp window (transposes(1,2) are
            # emitted during combo 14)
            fillers.setdefault(combos[15], []).extend(
                ("op", 1, jc, 2) for jc in range(8))
            prev = (0, 0, 0)
            pts_prev = pts0
            for c in combos[1:]:
                load_mask(c[0], c[1])
                pts = qk_exp(*c)
                for f in fillers.get(c, []):
                    if f[0] == "q":
                        q_part(f[1])
                    elif f[0] == "kv":
                        kv_part(f[1])
                    else:
                        outproj_quarter(f[1], f[2], f[3])
                if prev is not None:
                    pv_norm(*prev, pts_prev)
                    if prev[2] == HPC - 1:
                        transposes(prev[0], prev[1])
                prev, pts_prev = c, pts
            pv_norm(*prev, pts_prev)
            transposes(prev[0], prev[1])
            for jc in range(8):
                outproj_quarter(1, jc, 3, evac_act=True)

    nc.compile()
    return nc


# ---------------------------------------------------------------- pjrt runner
def _make_runner(nc, n_cores=NCORES):
    import jax
    from jax.sharding import Mesh, PartitionSpec
    from jax.experimental.shard_map import shard_map
    from concourse.bass2jax import (_bass_exec_p, install_neuronx_cc_hook,
                                    partition_id_tensor)

    install_neuronx_cc_hook()
    partition_name = (nc.partition_id_tensor.name
                      if nc.partition_id_tensor else None)
    in_names, out_names, out_avals, zero_shapes = [], [], [], []
    for alloc in nc.m.functions[0].allocations:
        if not isinstance(alloc, mybir.MemoryLocationSet):
            continue
        name = alloc.memorylocations[0].name
        if alloc.kind == "ExternalInput":
            if name != partition_name:
                in_names.append(name)
        elif alloc.kind == "ExternalOutput":
            shape = tuple(alloc.tensor_shape)
            dtype = mybir.dt.np(alloc.dtype)
            out_names.append(name)
            out_avals.append(jax.core.ShapedArray(shape, dtype))
            zero_shapes.append((shape, dtype))
    n_params = len(in_names)
    n_outs = len(out_avals)
    all_in_names = list(in_names) + list(out_names)
    if partition_name is not None:
        all_in_names.append(partition_name)

    def _body(*args):
        operands = list(args)
        if partition_name is not None:
            operands.append(partition_id_tensor())
        return tuple(_bass_exec_p.bind(
            *operands, out_avals=tuple(out_avals), in_names=tuple(all_in_names),
            out_names=tuple(out_names), lowering_input_output_aliases=(),
            sim_require_finite=True, sim_require_nnan=True, nc=nc))

    devices = jax.devices()[:n_cores]
    mesh = Mesh(np.asarray(devices), ("core",))
    in_specs = (PartitionSpec("core"),) * (n_params + n_outs)
    out_specs = (PartitionSpec("core"),) * len(out_names)
    donate = tuple(range(n_params, n_params + n_outs))
    sharded = jax.jit(
        shard_map(_body, mesh=mesh, in_specs=in_specs, out_specs=out_specs,
                  check_rep=False),
        donate_argnums=donate, keep_unused=True)

    def run(in_maps, time_iters=0):
        per_core = [[np.asarray(m[name]) for name in in_names]
                    for m in in_maps]
        concat_in = [np.concatenate([per_core[c][i] for c in range(n_cores)],
                                    axis=0) for i in range(n_params)]

        def zeros():
            return [np.zeros((n_cores * s[0], *s[1:]), d)
                    for s, d in zero_shapes]

        import jax
        out_arrs = sharded(*concat_in, *zeros())
        jax.block_until_ready(out_arrs)
        times = []
        for _ in range(time_iters):
            t0 = time.perf_counter()
            o = sharded(*concat_in, *zeros())
            jax.block_until_ready(o)
            times.append(time.perf_counter() - t0)
            out_arrs = o
        results = [
            {name: np.asarray(out_arrs[i]).reshape(n_cores,
                                                   *out_avals[i].shape)[c]
             for i, name in enumerate(out_names)}
            for c in range(n_cores)]
        return results, times

    return run


# ---------------------------------------------------------------- host shard
def _tables(positions, core, npos_dtype=np.float64):
    """cos/sin [128, T] fp16 tables; sign of the splice folded into sin."""
    pos = np.asarray(positions, np.float64).T.reshape(T)   # b-major tokens
    nb = NROT // 2
    freq = MAX_WL ** (2.0 / NROT * np.linspace(0.0, float(nb), nb))
    inv = 1.0 / freq                                        # [16]
    cos = np.ones((128, T), np.float64)
    sin = np.zeros((128, T), np.float64)
    for hl in range(HPC):
        hglob = core * HPC + hl
        if hglob >= NHEADS_ROT:
            continue
        for cc in range(NROT):
            ang = pos * inv[cc // 2]
            r = hl * DH + cc
            cos[r] = np.cos(ang)
            sgn = -1.0 if cc % 2 == 0 else 1.0
            sin[r] = sgn * np.sin(ang)
    return cos.astype(NPF16), sin.astype(NPF16)


def _pack_tab(cos, sin):
    """[cos_blk | sin_blk] interleaved per 512-token block."""
    chunks = []
    for tb in range(TB):
        chunks.append(cos[:, tb * 512:(tb + 1) * 512])
        chunks.append(sin[:, tb * 512:(tb + 1) * 512])
    return np.ascontiguousarray(np.concatenate(chunks, axis=1))


def make_in_maps(inputs_q, inputs_kv, mask, q_positions, kv_positions,
                 Wq, bq, Wk, bk, Wv, bv, Wo, bo, use_mask):
    f32 = np.float32
    xqT = np.ascontiguousarray(
        np.asarray(inputs_q, f32).transpose(2, 1, 0).reshape(D, T)).astype(NPF16)
    xkvT = np.ascontiguousarray(
        np.asarray(inputs_kv, f32).transpose(2, 1, 0).reshape(D, T)).astype(NPF16)
    scale = f32(1.0 / np.sqrt(DH))
    Wq, Wk, Wv, Wo = (np.asarray(a, f32) for a in (Wq, Wk, Wv, Wo))
    bq, bk, bv, bo = (np.asarray(a, f32) for a in (bq, bk, bv, bo))
    iden = np.eye(128, dtype=NPF16)
    perm = np.zeros((128, 128), NPF16)
    perm[np.arange(128), np.arange(128) ^ 1] = 1.0
    idpm = np.concatenate([iden, perm], axis=1)
    if use_mask:
        maskT = np.ascontiguousarray((np.asarray(mask) > 0).astype(NPF16))

    in_maps = []
    for c in range(NCORES):
        sl = slice(c * MPC, (c + 1) * MPC)
        cq, sq = _tables(q_positions, c)
        ck, sk = _tables(kv_positions, c)
        w3 = np.concatenate(
            [Wk[sl, :].T, Wv[sl, :].T, (scale * Wq[sl, :]).T],
            axis=1)
        bqk = np.stack([bk[sl], scale * bq[sl]], axis=1)
        m = {
            "xqT": xqT, "xkvT": xkvT,
            "w3": np.ascontiguousarray(w3).astype(NPF16),
            "bqk": np.ascontiguousarray(bqk, np.float32),
            "woT": np.ascontiguousarray(Wo[:, sl].T).astype(NPF16),
            "tabq": _pack_tab(cq, sq),
            "tabk": _pack_tab(ck, sk),
            "idpm": idpm,
        }
        if use_mask:
            m["maskT"] = maskT
        in_maps.append(m)
    return in_maps


_CACHE = {}


def _get(use_mask):
    if use_mask not in _CACHE:
        nc = build_kernel(use_mask)
        _CACHE[use_mask] = (nc, _make_runner(nc))
    return _CACHE[use_mask]


def kernel(inputs_q, inputs_kv, mask, q_positions, kv_positions,
           Wq, bq, Wk, bk, Wv, bv, Wo, bo, _time_iters=0):
    use_mask = not bool(np.all(np.asarray(mask) > 0))
    nc, run = _get(use_mask)
    in_maps = make_in_maps(inputs_q, inputs_kv, mask, q_positions,
                           kv_positions, Wq, bq, Wk, bk, Wv, bv, Wo, bo,
                           use_mask)
    results, times = run(in_maps, time_iters=_time_iters)
    acc = np.zeros((D, T), np.float64)
    for c in range(NCORES):
        acc += results[c]["outT"].astype(np.float64)
    bo_full = (np.asarray(Wo, np.float64) @ np.asarray(bv, np.float64)
               + np.asarray(bo, np.float64))
    acc += bo_full[:, None]
    out = acc.astype(np.float32).reshape(D, B, TQ).transpose(2, 1, 0)
    out = np.ascontiguousarray(out)
    if _time_iters:
        kernel._last_times = times
    return out
